# revision 26
# baseline (speedup 1.0000x reference)
"""Trainium2 kernel for nn_BernNet_47364899340878.

Math note (why the device kernel is just the MLP):
  The reference computes  out = sum_{j=0..K} c_j * relu(temp_j) * L^j (2I-L)^{K-j} h
  with c_j = C(K,j)/2^K and h = relu(x@W1+b1)@W2+b2.  The graded inputs pin
  temp = ones (spec fill "ones"), so relu(temp_j) = 1 for all j.  L and
  (2I - L) are commuting polynomials in the normalized adjacency, so the
  binomial theorem gives

      sum_j C(K,j) L^j (2I-L)^{K-j} = (L + 2I - L)^K = (2I)^K = 2^K I,

  i.e. the whole K=10 Bernstein propagation is exactly the identity map and
  out == h.  A non-ones temp (never the case for the graded inputs) falls
  back to a host implementation of the propagation for correctness.

Device kernel: h = relu(x@W1+b1)@W2+b2 and log_softmax(h), row-sharded over
8 NeuronCores (12500 rows each).  The kernel is HBM-bandwidth bound; traffic
per core is ~12.8 MB in + ~1.1 MB out (~39 us roofline at 358 GB/s):
  - every DMA uses ALL 128 SBUF partitions: profiling showed the HWDGE
    splits a transfer across SDMA engines by dividing the partition count
    evenly (largest divisor <= 16), so 125-partition transfers ran on only
    5 of 16 engines (111 GB/s ceiling) while 128-partition ones use all 16
    (~300-360 GB/s).  The contraction is host-padded 500 -> 512 = 4 x 128,
  - x streams as bf16, two 500-row blocks per DMA, in a host-prepped layout
    [pair, p(128), kc(4), r(1000)] (8000B contiguous run per partition),
  - only logp ships from the device (bf16, 4 blocks per DMA), plus one tiny
    fp32 lse tensor at the end; the host reconstructs the raw logits as
    out = logp + lse exactly,
  - compute runs as a per-block software pipeline S1 mm1 -> S2 bias+relu ->
    S3 mm2 -> S4 softmax-epilogue with S2/S3 one block behind S1 and S4 two
    behind, so each engine's FIFO always has the PE-gating work first,
  - mm1 accumulates into two 250-column PSUM half-tiles in separate banks;
    the bias+relu halves run on DIFFERENT engines (ACT Relu-with-bias for
    half 1, DVE tensor_scalar for half 2) and write separate SBUF tiles so
    the PE's mm2 subtiles never wait on the busier engine,
  - logp output DMAs issue from the otherwise-idle GpSimd (SWDGE) so their
    ~1us issue cost never delays ACT's relu/exp queue,
  - Exp, Ln and Relu are pinned to their shared ACT table set so the whole
    kernel does one table load,
  - DMA/compute startup is ordered so the first matmul's semaphore wait
    (which covers every earlier-emitted input DMA) only gates on w1 + the
    leftover block: w1, xl, small weights, 3 HAM warm-up matmuls, block 24's
    matmuls, and only then the streaming pair DMAs.
Bias handling folds into the matmuls: W1 gains a 65th output column of
zeros whose bias is 1.0 so h^T gets a row of ones, and W2 gains a 65th
input row equal to b2.  (If b2 != 0 AND temp != ones the host fallback
recomputes exactly; for graded inputs b2 = 0.)
Numeric error vs the fp32 reference is ~7e-3 absmax-rel (gate 2e-2).
"""

import numpy as np

_N = 100000
_FIN = 500
_FPAD = 512  # contraction padded to 4 chunks x 128 partitions
_HID = 64
_CLS = 40
_NCORES = 8
_RPC = _N // _NCORES  # 12500 rows per core
_P = 128  # contraction partitions per chunk
_KC = 4  # contraction chunks
_BLK = 500  # rows per block
_NBLK = _RPC // _BLK  # 25
_NPAIR = 12  # paired input DMAs; block 24 is the leftover
_NQUAD = 6  # output DMAs of 4 blocks each (blocks 0..23)
_SUB = 125  # rows per mm2 subtile
_NSUB = 4
_HPAD = 504  # h^T tile columns: 500 rows + 4 zero pad (128-col mm2 slices)

_CACHE = {}


def _build_bass():
    """Build the per-core Bass program (shared by all 8 cores)."""
    from contextlib import ExitStack

    import concourse.bacc as bacc
    import concourse.mybir as mybir
    import concourse.tile as tile

    fp32 = mybir.dt.float32
    bf16 = mybir.dt.bfloat16
    AF = mybir.ActivationFunctionType
    OP = mybir.AluOpType

    # Bacc (not plain Bass): its compile() runs move_matmul_waits_to_ldweights
    # + generate_event_semaphores, which split excess on_wait entries to meet
    # TRN2's 1-wait-per-instruction constraint that walrus enforces.
    #
    # Table-set pinning: ACT function tables are loaded as named sets and a
    # set switch costs ~1.3-2.7us.  Exp and Ln both live in the
    # "natural_log_exp_and_others" set, but the default insertion pass picks
    # each function's first containing set, so an Exp/Ln mix reloads on every
    # switch.  Restricting Exp/Ln to their shared set (keeping every set's
    # positional id intact) makes the whole kernel need exactly one load.
    class _PinnedActBacc(bacc.Bacc):
        def insert_act_table_loads(self):
            import bass_rust as _bass_rust
            from concourse.hw_specs import get_activation_tables

            has_activation = any(
                isinstance(i, mybir.InstActivation)
                for b in self.main_func.blocks
                for i in b.instructions
            )
            if not has_activation:
                return
            shared = {AF.Exp, AF.Ln, AF.Relu}
            tables = []
            for name, fns in get_activation_tables(self.m.arch).items():
                if name != "natural_log_exp_and_others":
                    fns = fns - shared
                tables.append((name, fns))
            _bass_rust.insert_act_table_loads(self, tables)

    nc = _PinnedActBacc()
    xt = nc.dram_tensor("xt", [_NBLK, _P, _KC, _BLK], bf16, kind="ExternalInput")
    w1 = nc.dram_tensor("w1", [_P, _KC, 128], bf16, kind="ExternalInput")
    b1 = nc.dram_tensor("b1", [_HID + 1, 1], fp32, kind="ExternalInput")
    w2 = nc.dram_tensor("w2", [_HID + 1, _CLS], bf16, kind="ExternalInput")
    # logp quads: [quad, p, kq(block-in-quad), si, c] bf16 — each partition's
    # quad data is one contiguous 1280B DRAM run, split into 640B descriptors.
    both = nc.dram_tensor(
        "both", [_NQUAD, _P, 4, _NSUB, _CLS], bf16, kind="ExternalOutput"
    )
    last = nc.dram_tensor("last", [_P, _NSUB, _CLS], bf16, kind="ExternalOutput")
    lse_d = nc.dram_tensor("lse", [_P, _NBLK, _NSUB], fp32, kind="ExternalOutput")

    xt_r = xt.rearrange("b p kc r -> b p kc r")
    both_r = both.rearrange("q p k si c -> q p k si c")

    with tile.TileContext(nc) as tc, ExitStack() as ctx:
        const = ctx.enter_context(tc.tile_pool(name="const", bufs=1))
        xpool = ctx.enter_context(tc.tile_pool(name="xin", bufs=5))
        hpool = ctx.enter_context(tc.tile_pool(name="hrelu", bufs=3))
        epool = ctx.enter_context(tc.tile_pool(name="expv", bufs=3))
        cpool = ctx.enter_context(tc.tile_pool(name="outs", bufs=2))
        spool = ctx.enter_context(tc.tile_pool(name="sums", bufs=3))
        pp1a = ctx.enter_context(tc.tile_pool(name="ps1a", bufs=2, space="PSUM"))
        pp1b = ctx.enter_context(tc.tile_pool(name="ps1b", bufs=2, space="PSUM"))
        pp2 = ctx.enter_context(tc.tile_pool(name="ps2", bufs=3, space="PSUM"))
        ppw = ctx.enter_context(tc.tile_pool(name="psw", bufs=1, space="PSUM"))

        # Issue order matters doubly here: the SP sequencer takes ~1us to
        # issue each DMA, and the first matmul's semaphore wait covers every
        # input DMA emitted before it.  So: w1 first (it lands ~1.7us before
        # xl and unblocks the warm-up matmuls), then xl (block 24's data),
        # then w2/b1 (needed one pipeline stage later); the streaming pair
        # DMAs are emitted only after block 24's matmuls.
        w1_sb = const.tile([_P, _KC, 128], bf16)
        nc.sync.dma_start(w1_sb[:], w1[:])
        blk_sb = {}
        blk_sb[0] = xpool.tile([_P, _KC, _BLK], bf16, tag="xt", name="xtb0")
        nc.sync.dma_start(blk_sb[0][:], xt_r[0])
        w2_sb = const.tile([_HID + 1, _CLS], bf16)
        nc.sync.dma_start(w2_sb[:], w2[:])
        b1_sb = const.tile([_HID + 1, 1], fp32)
        nc.sync.dma_start(b1_sb[:], b1[:])
        lse_sb = const.tile([_P, _NBLK, _NSUB], fp32)

        # ~2us of dummy matmuls gated only on w1 (which lands ~1.7us before
        # xl): the PE would otherwise sit idle, and this pre-warms the HAM
        # clock before block 24's real matmuls start.
        warm_ps = ppw.tile([128, _KC, 128], fp32)
        for i in range(3):
            nc.tensor.matmul(warm_ps[:], w1_sb[:, 0, :], w1_sb[:])



        def mm1_block(xt_sb):
            # h^T = (W1p^T @ x^T) : [128(65 live), 500], accumulated over 4
            # K-chunks, split into two 250-row column halves in SEPARATE PSUM
            # banks so the bias+relu of half 1 can run while half 2's matmuls
            # are still streaming (PSUM bank collision rules forbid reading a
            # bank the PE is writing).
            h1 = pp1a.tile([128, _BLK // 2], fp32)
            h2 = pp1b.tile([128, _BLK // 2], fp32)
            for half, hp in ((0, h1), (1, h2)):
                lo = half * (_BLK // 2)
                for kc in range(_KC):
                    nc.tensor.matmul(
                        hp[:],
                        w1_sb[:, kc, :],
                        xt_sb[:, kc, lo : lo + _BLK // 2],
                        start=(kc == 0),
                        stop=(kc == _KC - 1),
                    )
            return h1, h2

        def relu_block(h1, h2):
            # fused bias+relu, one half per engine (DVE tensor_scalar + ACT
            # Relu-with-bias) so neither engine's queue gates the PE's mm2;
            # row 64 = max(0+1,0) = 1 (the bias-ones row).  Relu shares the
            # pinned ACT table set with Exp/Ln, so no table reloads.  The two
            # halves are SEPARATE tiles so mm2's first subtiles depend only
            # on the DVE half, not on whichever engine finishes last.
            # ACT takes half 1 (mm2's si0/si1 need it only after the whole
            # next mm1 block, so ACT's exp/ln/dma queue can't hurt); DVE takes
            # half 2, which gates si2/si3 soonest after mm1 completes.
            r1 = hpool.tile([_HID + 1, _BLK // 2], bf16, tag="ht1")
            nc.scalar.activation(
                r1[:], h1[: _HID + 1, :], AF.Relu, bias=b1_sb[:],
            )
            r2 = hpool.tile([_HID + 1, _BLK // 2], bf16, tag="ht2")
            nc.vector.tensor_scalar(
                out=r2[:], in0=h2[: _HID + 1, :],
                scalar1=b1_sb[:], scalar2=0.0, op0=OP.add, op1=OP.max,
            )
            return r1, r2

        def mm2_block(relus):
            # out = h_relu_aug^T.T @ W2_aug : 4 subtiles of 125 rows, two per
            # relu half (LDWEIGHTS are pulled ahead by the PE reorder window,
            # so the per-subtile stationary reload is fully hidden)
            r1, r2 = relus
            o_ps = pp2.tile([_SUB, _NSUB, _CLS], fp32)
            for si in range(_NSUB):
                src_t = r1 if si < 2 else r2
                lo = (si % 2) * _SUB
                nc.tensor.matmul(
                    o_ps[:, si, :],
                    src_t[:, lo : lo + _SUB],
                    w2_sb[:],
                )
            return o_ps

        def epi_block(o_ps, b, cmb_slot):
            # log_softmax without max-subtraction (logits bounded |h| < ~6 so
            # exp cannot overflow).  exp/sub read PSUM directly.  Only lanes
            # 0..124 are live; the output tiles' lanes 125..127 were zeroed
            # once per pool slot so the 128-partition DMAs ship finite data.
            e_sb = epool.tile([_SUB, _NSUB, _CLS], fp32)
            nc.scalar.activation(e_sb[:], o_ps[:], AF.Exp)
            ssum = spool.tile([_SUB, _NSUB], fp32)
            nc.vector.tensor_reduce(
                out=ssum[:], in_=e_sb[:], op=OP.add, axis=mybir.AxisListType.X,
            )
            nc.scalar.activation(lse_sb[:_SUB, b, :], ssum[:], AF.Ln)
            nc.vector.tensor_sub(
                cmb_slot[:_SUB],
                o_ps[:],
                lse_sb[:_SUB, b, :, None].broadcast_to([_SUB, _NSUB, _CLS]),
            )

        # Software pipeline over blocks, leftover block 24 first (its data
        # lands before pair 0, and processing it first keeps the tail short).
        # Stages per block: S1 mm1 -> S2 bias+relu -> S3 mm2 -> S4 softmax
        # epilogue, with S2/S3 one block behind S1 and S4 two behind.  This
        # keeps the DVE relu (which gates the PE's mm2) ahead of the longer
        # softmax chain in the DVE FIFO, and sandwiches each mm2 between
        # mm1s so the PE never head-of-line blocks on the relu.
        seq = list(range(_NBLK))
        cmb_last = cpool.tile([_P, _NSUB, _CLS], bf16, tag="cl")
        nc.vector.memset(cmb_last[96:], 0.0)
        nc.vector.memset(lse_sb[96:], 0.0)
        cmb_quad = {}

        def cmb_slot(b):
            if b == _NBLK - 1:
                return cmb_last[:]
            q = b // 4
            if q not in cmb_quad:
                cmb_quad[q] = cpool.tile([_P, 4, _NSUB, _CLS], bf16, tag="cq", name=f"cmbq{q}")
                if q < 2:
                    # cpool slots cycle round-robin; zero the dead lanes of
                    # each slot once so every later quad ships finite data
                    nc.vector.memset(cmb_quad[q][96:], 0.0)
            return cmb_quad[q][:, b % 4]

        ht_ps_of = {}
        o_ps_of = {}

        def stage1(b):
            # mm1 FIRST, prefetch DMAs after: the first matmul of a block is
            # semaphore-gated on every input DMA emitted before it, so the
            # lookahead transfers must sit after it in program order.
            ht_ps_of[b] = mm1_block(blk_sb[b])
            blk_sb.pop(b)
            hi = min(b + 3, _NBLK - 1)
            for nb in range(b + 1, hi + 1):
                if nb not in blk_sb:
                    t = xpool.tile([_P, _KC, _BLK], bf16, tag="xt", name=f"xtb{nb}")
                    nc.sync.dma_start(t[:], xt_r[nb])
                    blk_sb[nb] = t

        def stage23(b, idx):
            h1, h2 = ht_ps_of.pop(b)
            o_ps_of[b] = mm2_block(relu_block(h1, h2))

        def stage4(b):
            epi_block(o_ps_of.pop(b), b, cmb_slot(b))
            if b == _NBLK - 1:
                nc.gpsimd.dma_start(last[:], cmb_last[:])
            elif b % 4 == 3:
                # one DMA per quad via GpSimd SWDGE — the engine is idle, so
                # the ~1us descriptor-issue cost never delays ACT's relu/exp
                # queue; [128, 1280B] spreads all 16 SDMA engines
                nc.gpsimd.dma_start(both_r[b // 4], cmb_quad.pop(b // 4)[:])

        for idx, b in enumerate(seq):
            stage1(b)
            if idx >= 1:
                stage23(seq[idx - 1], idx - 1)
            if idx >= 2:
                stage4(seq[idx - 2])
        stage23(seq[-1], len(seq) - 1)
        stage4(seq[-2])
        stage4(seq[-1])

        # ship the lse accumulator once at the end (64KB)
        nc.gpsimd.dma_start(lse_d[:], lse_sb[:])

    nc.finalize()
    return nc


def _get_bass():
    if "nc" not in _CACHE:
        _CACHE["nc"] = _build_bass()
    return _CACHE["nc"]


def _host_prep(x, W1, b1, W2, b2):
    """Weights/bias in device layout (bf16, bias-augmented, FWL/DMA-padded)."""
    import ml_dtypes

    bf = ml_dtypes.bfloat16
    x = np.asarray(x, np.float32)
    x_bf = np.zeros((x.shape[0], _FPAD), bf)
    x_bf[:, :_FIN] = x.astype(bf)  # [N, 512]
    w1p = np.zeros((_P, _KC, 128), bf)
    W1b = np.zeros((_FPAD, _HID), bf)
    W1b[:_FIN] = np.asarray(W1, np.float32).astype(bf)
    # feature f = kc*128 + p  ->  w1p[p, kc, m]
    w1p[:, :, :_HID] = W1b.reshape(_KC, _P, _HID).transpose(1, 0, 2)
    b1a = np.zeros((_HID + 1, 1), np.float32)
    b1a[:_HID, 0] = np.asarray(b1, np.float32)
    b1a[_HID, 0] = 1.0
    w2a = np.zeros((_HID + 1, _CLS), bf)
    w2a[:_HID] = np.asarray(W2, np.float32).astype(bf)
    w2a[_HID] = np.asarray(b2, np.float32).astype(bf)
    return x_bf, w1p, b1a, w2a


def _core_x(x_bf, c):
    """Per-core input in device layout [blk, p, kc, r] (4000B runs)."""
    xs = x_bf[c * _RPC : (c + 1) * _RPC]  # [12500, 512]
    # row = blk*500 + r ; feature = kc*128 + p
    return np.ascontiguousarray(
        xs.reshape(_NBLK, _BLK, _KC, _P).transpose(0, 3, 2, 1)
    )


def _in_maps(x, W1, b1, W2, b2):
    x_bf, w1p, b1a, w2a = _host_prep(x, W1, b1, W2, b2)
    return [
        {"xt": _core_x(x_bf, c), "w1": w1p, "b1": b1a, "w2": w2a}
        for c in range(_NCORES)
    ]


def _unshard(res):
    outs = []
    lps = []
    for c in range(_NCORES):
        a = np.asarray(res.results[c]["both"])[:, :_SUB].astype(np.float32)
        l = np.asarray(res.results[c]["last"])[:_SUB].astype(np.float32)
        lse = np.asarray(res.results[c]["lse"])[:_SUB].astype(np.float32)
        # a[q, p, kq, si, c] -> rows (q, kq, si, p)
        la = a.transpose(0, 2, 3, 1, 4).reshape(_NQUAD * 4 * _BLK, _CLS)
        # l[p, si, c] -> rows (si, p)
        ll = l.transpose(1, 0, 2).reshape(_BLK, _CLS)
        lp = np.concatenate([la, ll])  # [12500, 40] logp
        # lse[p, b, si] -> row b*500 + si*125 + p
        lse_rows = lse.transpose(1, 2, 0).reshape(_RPC)
        out = lp + lse_rows[:, None]
        lps.append(lp)
        outs.append(out)
    return np.concatenate(lps), np.concatenate(outs)


def _bern_prop_host(h, edge_index, theta):
    """Fallback: full Bernstein propagation on host (only if temp != ones)."""
    from math import comb

    n = h.shape[0]
    src = np.asarray(edge_index[0], np.int64)
    dst = np.asarray(edge_index[1], np.int64)
    deg = np.bincount(src, minlength=n).astype(np.float32)
    dis = np.where(deg > 0, 1.0 / np.sqrt(np.maximum(deg, 1.0)), 0.0).astype(
        np.float32
    )

    def anorm(v):
        msg = v[src] * dis[src][:, None]
        out = np.zeros_like(v)
        np.add.at(out, dst, msg)
        return out * dis[:, None]

    K = len(theta) - 1
    tmp = [h]
    for _ in range(K):
        t = tmp[-1]
        tmp.append(t + anorm(t))
    c = np.array([comb(K, j) / 2.0**K for j in range(K + 1)], np.float32)
    acc = np.zeros_like(h)
    for j in range(K, 0, -1):
        s = acc + c[j] * theta[j] * tmp[K - j]
        acc = s - anorm(s)
    return c[0] * theta[0] * tmp[K] + acc


def kernel(x, edge_index, W1, b1, W2, b2, temp):
    from concourse.bass_utils import run_bass_kernel_spmd

    nc = _get_bass()
    in_maps = _in_maps(x, W1, b1, W2, b2)
    res = run_bass_kernel_spmd(nc, in_maps, core_ids=list(range(_NCORES)))
    lp, out = _unshard(res)

    theta = np.maximum(np.asarray(temp, np.float32), 0.0)
    if not np.allclose(theta, 1.0):
        # General-temp path: device computed h; propagate on host, then
        # recompute log_softmax.
        out = _bern_prop_host(out.astype(np.float32), edge_index, theta)
        m = out.max(axis=1, keepdims=True)
        lp = out - (np.log(np.exp(out - m).sum(axis=1, keepdims=True)) + m)
        lp = lp.astype(np.float32)

    return lp, out


# revision 27
# speedup vs baseline: 1.0076x; 1.0076x over previous
"""Trainium2 kernel for nn_BernNet_47364899340878.

Math note (why the device kernel is just the MLP):
  The reference computes  out = sum_{j=0..K} c_j * relu(temp_j) * L^j (2I-L)^{K-j} h
  with c_j = C(K,j)/2^K and h = relu(x@W1+b1)@W2+b2.  The graded inputs pin
  temp = ones (spec fill "ones"), so relu(temp_j) = 1 for all j.  L and
  (2I - L) are commuting polynomials in the normalized adjacency, so the
  binomial theorem gives

      sum_j C(K,j) L^j (2I-L)^{K-j} = (L + 2I - L)^K = (2I)^K = 2^K I,

  i.e. the whole K=10 Bernstein propagation is exactly the identity map and
  out == h.  A non-ones temp (never the case for the graded inputs) falls
  back to a host implementation of the propagation for correctness.

Device kernel: h = relu(x@W1+b1)@W2+b2 and log_softmax(h), row-sharded over
8 NeuronCores (12500 rows each).  The kernel is HBM-bandwidth bound; traffic
per core is ~12.8 MB in + ~1.1 MB out (~39 us roofline at 358 GB/s):
  - every DMA uses ALL 128 SBUF partitions: profiling showed the HWDGE
    splits a transfer across SDMA engines by dividing the partition count
    evenly (largest divisor <= 16), so 125-partition transfers ran on only
    5 of 16 engines (111 GB/s ceiling) while 128-partition ones use all 16
    (~300-360 GB/s).  The contraction is host-padded 500 -> 512 = 4 x 128,
  - x streams as bf16, two 500-row blocks per DMA, in a host-prepped layout
    [pair, p(128), kc(4), r(1000)] (8000B contiguous run per partition),
  - only logp ships from the device (bf16, 4 blocks per DMA), plus one tiny
    fp32 lse tensor at the end; the host reconstructs the raw logits as
    out = logp + lse exactly,
  - compute runs as a per-block software pipeline S1 mm1 -> S2 bias+relu ->
    S3 mm2 -> S4 softmax-epilogue with S2/S3 one block behind S1 and S4 two
    behind, so each engine's FIFO always has the PE-gating work first,
  - mm1 accumulates into two 250-column PSUM half-tiles in separate banks;
    the bias+relu halves run on DIFFERENT engines (ACT Relu-with-bias for
    half 1, DVE tensor_scalar for half 2) and write separate SBUF tiles so
    the PE's mm2 subtiles never wait on the busier engine,
  - logp output DMAs issue from the otherwise-idle GpSimd (SWDGE) so their
    ~1us issue cost never delays ACT's relu/exp queue,
  - Exp, Ln and Relu are pinned to their shared ACT table set so the whole
    kernel does one table load,
  - DMA/compute startup is ordered so the first matmul's semaphore wait
    (which covers every earlier-emitted input DMA) only gates on w1 + the
    leftover block: w1, xl, small weights, 3 HAM warm-up matmuls, block 24's
    matmuls, and only then the streaming pair DMAs.
Bias handling folds into the matmuls: W1 gains a 65th output column of
zeros whose bias is 1.0 so h^T gets a row of ones, and W2 gains a 65th
input row equal to b2.  (If b2 != 0 AND temp != ones the host fallback
recomputes exactly; for graded inputs b2 = 0.)
Numeric error vs the fp32 reference is ~7e-3 absmax-rel (gate 2e-2).
"""

import numpy as np

_N = 100000
_FIN = 500
_FPAD = 512  # contraction padded to 4 chunks x 128 partitions
_HID = 64
_CLS = 40
_NCORES = 8
_RPC = _N // _NCORES  # 12500 rows per core
_P = 128  # contraction partitions per chunk
_KC = 4  # contraction chunks
_BLK = 500  # rows per block
_NBLK = _RPC // _BLK  # 25
_NPAIR = 12  # paired input DMAs; block 24 is the leftover
_NQUAD = 6  # output DMAs of 4 blocks each (blocks 0..23)
_SUB = 125  # rows per mm2 subtile
_NSUB = 4
_HPAD = 504  # h^T tile columns: 500 rows + 4 zero pad (128-col mm2 slices)

_CACHE = {}


def _build_bass():
    """Build the per-core Bass program (shared by all 8 cores)."""
    from contextlib import ExitStack

    import concourse.bacc as bacc
    import concourse.mybir as mybir
    import concourse.tile as tile

    fp32 = mybir.dt.float32
    bf16 = mybir.dt.bfloat16
    AF = mybir.ActivationFunctionType
    OP = mybir.AluOpType

    # Bacc (not plain Bass): its compile() runs move_matmul_waits_to_ldweights
    # + generate_event_semaphores, which split excess on_wait entries to meet
    # TRN2's 1-wait-per-instruction constraint that walrus enforces.
    #
    # Table-set pinning: ACT function tables are loaded as named sets and a
    # set switch costs ~1.3-2.7us.  Exp and Ln both live in the
    # "natural_log_exp_and_others" set, but the default insertion pass picks
    # each function's first containing set, so an Exp/Ln mix reloads on every
    # switch.  Restricting Exp/Ln to their shared set (keeping every set's
    # positional id intact) makes the whole kernel need exactly one load.
    class _PinnedActBacc(bacc.Bacc):
        def insert_act_table_loads(self):
            import bass_rust as _bass_rust
            from concourse.hw_specs import get_activation_tables

            has_activation = any(
                isinstance(i, mybir.InstActivation)
                for b in self.main_func.blocks
                for i in b.instructions
            )
            if not has_activation:
                return
            shared = {AF.Exp, AF.Ln, AF.Relu}
            tables = []
            for name, fns in get_activation_tables(self.m.arch).items():
                if name != "natural_log_exp_and_others":
                    fns = fns - shared
                tables.append((name, fns))
            _bass_rust.insert_act_table_loads(self, tables)

    nc = _PinnedActBacc()
    xt = nc.dram_tensor("xt", [_NBLK, _P, _KC, _BLK], bf16, kind="ExternalInput")
    w1 = nc.dram_tensor("w1", [_P, _KC, 128], bf16, kind="ExternalInput")
    b1 = nc.dram_tensor("b1", [_HID + 1, 1], fp32, kind="ExternalInput")
    w2 = nc.dram_tensor("w2", [_HID + 1, _CLS], bf16, kind="ExternalInput")
    # logp quads: [quad, p, kq(block-in-quad), si, c] bf16 — each partition's
    # quad data is one contiguous 1280B DRAM run, split into 640B descriptors.
    both = nc.dram_tensor(
        "both", [_NQUAD, _P, 4, _NSUB, _CLS], bf16, kind="ExternalOutput"
    )
    last = nc.dram_tensor("last", [_P, _NSUB, _CLS], bf16, kind="ExternalOutput")
    lse_d = nc.dram_tensor("lse", [_P, _NBLK, _NSUB], fp32, kind="ExternalOutput")

    xt_r = xt.rearrange("b p kc r -> b p kc r")
    both_r = both.rearrange("q p k si c -> q p k si c")

    with tile.TileContext(nc) as tc, ExitStack() as ctx:
        const = ctx.enter_context(tc.tile_pool(name="const", bufs=1))
        xpool = ctx.enter_context(tc.tile_pool(name="xin", bufs=8))
        hpool = ctx.enter_context(tc.tile_pool(name="hrelu", bufs=3))
        epool = ctx.enter_context(tc.tile_pool(name="expv", bufs=3))
        cpool = ctx.enter_context(tc.tile_pool(name="outs", bufs=2))
        spool = ctx.enter_context(tc.tile_pool(name="sums", bufs=3))
        pp1a = ctx.enter_context(tc.tile_pool(name="ps1a", bufs=2, space="PSUM"))
        pp1b = ctx.enter_context(tc.tile_pool(name="ps1b", bufs=2, space="PSUM"))
        pp2 = ctx.enter_context(tc.tile_pool(name="ps2", bufs=3, space="PSUM"))
        ppw = ctx.enter_context(tc.tile_pool(name="psw", bufs=1, space="PSUM"))

        # Issue order matters doubly here: the SP sequencer takes ~1us to
        # issue each DMA, and the first matmul's semaphore wait covers every
        # input DMA emitted before it.  So: w1 first (it lands ~1.7us before
        # xl and unblocks the warm-up matmuls), then xl (block 24's data),
        # then w2/b1 (needed one pipeline stage later); the streaming pair
        # DMAs are emitted only after block 24's matmuls.
        w1_sb = const.tile([_P, _KC, 128], bf16)
        nc.sync.dma_start(w1_sb[:], w1[:])
        blk_sb = {}
        blk_sb[0] = xpool.tile([_P, _KC, _BLK], bf16, tag="xt", name="xtb0")
        nc.sync.dma_start(blk_sb[0][:], xt_r[0])
        w2_sb = const.tile([_HID + 1, _CLS], bf16)
        nc.sync.dma_start(w2_sb[:], w2[:])
        b1_sb = const.tile([_HID + 1, 1], fp32)
        nc.sync.dma_start(b1_sb[:], b1[:])
        lse_sb = const.tile([_P, _NBLK, _NSUB], fp32)

        # ~2us of dummy matmuls gated only on w1 (which lands ~1.7us before
        # xl): the PE would otherwise sit idle, and this pre-warms the HAM
        # clock before block 24's real matmuls start.
        warm_ps = ppw.tile([128, _KC, 128], fp32)
        for i in range(3):
            nc.tensor.matmul(warm_ps[:], w1_sb[:, 0, :], w1_sb[:])



        def mm1_block(xt_sb):
            # h^T = (W1p^T @ x^T) : [128(65 live), 500], accumulated over 4
            # K-chunks, split into two 250-row column halves in SEPARATE PSUM
            # banks so the bias+relu of half 1 can run while half 2's matmuls
            # are still streaming (PSUM bank collision rules forbid reading a
            # bank the PE is writing).
            h1 = pp1a.tile([128, _BLK // 2], fp32)
            h2 = pp1b.tile([128, _BLK // 2], fp32)
            for half, hp in ((0, h1), (1, h2)):
                lo = half * (_BLK // 2)
                for kc in range(_KC):
                    nc.tensor.matmul(
                        hp[:],
                        w1_sb[:, kc, :],
                        xt_sb[:, kc, lo : lo + _BLK // 2],
                        start=(kc == 0),
                        stop=(kc == _KC - 1),
                    )
            return h1, h2

        def relu_block(h1, h2):
            # fused bias+relu, one half per engine (DVE tensor_scalar + ACT
            # Relu-with-bias) so neither engine's queue gates the PE's mm2;
            # row 64 = max(0+1,0) = 1 (the bias-ones row).  Relu shares the
            # pinned ACT table set with Exp/Ln, so no table reloads.  The two
            # halves are SEPARATE tiles so mm2's first subtiles depend only
            # on the DVE half, not on whichever engine finishes last.
            # ACT takes half 1 (mm2's si0/si1 need it only after the whole
            # next mm1 block, so ACT's exp/ln/dma queue can't hurt); DVE takes
            # half 2, which gates si2/si3 soonest after mm1 completes.
            r1 = hpool.tile([_HID + 1, _BLK // 2], bf16, tag="ht1")
            nc.scalar.activation(
                r1[:], h1[: _HID + 1, :], AF.Relu, bias=b1_sb[:],
            )
            r2 = hpool.tile([_HID + 1, _BLK // 2], bf16, tag="ht2")
            nc.vector.tensor_scalar(
                out=r2[:], in0=h2[: _HID + 1, :],
                scalar1=b1_sb[:], scalar2=0.0, op0=OP.add, op1=OP.max,
            )
            return r1, r2

        def mm2_block(relus):
            # out = h_relu_aug^T.T @ W2_aug : 4 subtiles of 125 rows, two per
            # relu half (LDWEIGHTS are pulled ahead by the PE reorder window,
            # so the per-subtile stationary reload is fully hidden)
            r1, r2 = relus
            o_ps = pp2.tile([_SUB, _NSUB, _CLS], fp32)
            for si in range(_NSUB):
                src_t = r1 if si < 2 else r2
                lo = (si % 2) * _SUB
                nc.tensor.matmul(
                    o_ps[:, si, :],
                    src_t[:, lo : lo + _SUB],
                    w2_sb[:],
                )
            return o_ps

        def epi_block(o_ps, b, cmb_slot):
            # log_softmax without max-subtraction (logits bounded |h| < ~6 so
            # exp cannot overflow).  exp/sub read PSUM directly.  Only lanes
            # 0..124 are live; the output tiles' lanes 125..127 were zeroed
            # once per pool slot so the 128-partition DMAs ship finite data.
            e_sb = epool.tile([_SUB, _NSUB, _CLS], fp32)
            nc.scalar.activation(e_sb[:], o_ps[:], AF.Exp)
            ssum = spool.tile([_SUB, _NSUB], fp32)
            nc.vector.tensor_reduce(
                out=ssum[:], in_=e_sb[:], op=OP.add, axis=mybir.AxisListType.X,
            )
            nc.scalar.activation(lse_sb[:_SUB, b, :], ssum[:], AF.Ln)
            nc.vector.tensor_sub(
                cmb_slot[:_SUB],
                o_ps[:],
                lse_sb[:_SUB, b, :, None].broadcast_to([_SUB, _NSUB, _CLS]),
            )

        # Software pipeline over blocks, leftover block 24 first (its data
        # lands before pair 0, and processing it first keeps the tail short).
        # Stages per block: S1 mm1 -> S2 bias+relu -> S3 mm2 -> S4 softmax
        # epilogue, with S2/S3 one block behind S1 and S4 two behind.  This
        # keeps the DVE relu (which gates the PE's mm2) ahead of the longer
        # softmax chain in the DVE FIFO, and sandwiches each mm2 between
        # mm1s so the PE never head-of-line blocks on the relu.
        seq = list(range(_NBLK))
        cmb_last = cpool.tile([_P, _NSUB, _CLS], bf16, tag="cl")
        nc.vector.memset(cmb_last[96:], 0.0)
        nc.vector.memset(lse_sb[96:], 0.0)
        cmb_quad = {}

        def cmb_slot(b):
            if b == _NBLK - 1:
                return cmb_last[:]
            q = b // 4
            if q not in cmb_quad:
                cmb_quad[q] = cpool.tile([_P, 4, _NSUB, _CLS], bf16, tag="cq", name=f"cmbq{q}")
                if q < 2:
                    # cpool slots cycle round-robin; zero the dead lanes of
                    # each slot once so every later quad ships finite data
                    nc.vector.memset(cmb_quad[q][96:], 0.0)
            return cmb_quad[q][:, b % 4]

        ht_ps_of = {}
        o_ps_of = {}

        def stage1(b):
            # mm1 FIRST, prefetch DMAs after: the first matmul of a block is
            # semaphore-gated on every input DMA emitted before it, so the
            # lookahead transfers must sit after it in program order.
            ht_ps_of[b] = mm1_block(blk_sb[b])
            blk_sb.pop(b)
            hi = min(b + 3, _NBLK - 1)
            for nb in range(b + 1, hi + 1):
                if nb not in blk_sb:
                    t = xpool.tile([_P, _KC, _BLK], bf16, tag="xt", name=f"xtb{nb}")
                    nc.sync.dma_start(t[:], xt_r[nb])
                    blk_sb[nb] = t

        def stage23(b, idx):
            h1, h2 = ht_ps_of.pop(b)
            o_ps_of[b] = mm2_block(relu_block(h1, h2))

        def stage4(b):
            epi_block(o_ps_of.pop(b), b, cmb_slot(b))
            if b == _NBLK - 1:
                nc.gpsimd.dma_start(last[:], cmb_last[:])
            elif b % 4 == 3:
                # one DMA per quad via GpSimd SWDGE — the engine is idle, so
                # the ~1us descriptor-issue cost never delays ACT's relu/exp
                # queue; [128, 1280B] spreads all 16 SDMA engines
                nc.gpsimd.dma_start(both_r[b // 4], cmb_quad.pop(b // 4)[:])

        for idx, b in enumerate(seq):
            stage1(b)
            if idx >= 1:
                stage23(seq[idx - 1], idx - 1)
            if idx >= 2:
                stage4(seq[idx - 2])
        stage23(seq[-1], len(seq) - 1)
        stage4(seq[-2])
        stage4(seq[-1])

        # ship the lse accumulator once at the end (64KB)
        nc.gpsimd.dma_start(lse_d[:], lse_sb[:])

    nc.finalize()
    return nc


def _get_bass():
    if "nc" not in _CACHE:
        _CACHE["nc"] = _build_bass()
    return _CACHE["nc"]


def _host_prep(x, W1, b1, W2, b2):
    """Weights/bias in device layout (bf16, bias-augmented, FWL/DMA-padded)."""
    import ml_dtypes

    bf = ml_dtypes.bfloat16
    x = np.asarray(x, np.float32)
    x_bf = np.zeros((x.shape[0], _FPAD), bf)
    x_bf[:, :_FIN] = x.astype(bf)  # [N, 512]
    w1p = np.zeros((_P, _KC, 128), bf)
    W1b = np.zeros((_FPAD, _HID), bf)
    W1b[:_FIN] = np.asarray(W1, np.float32).astype(bf)
    # feature f = kc*128 + p  ->  w1p[p, kc, m]
    w1p[:, :, :_HID] = W1b.reshape(_KC, _P, _HID).transpose(1, 0, 2)
    b1a = np.zeros((_HID + 1, 1), np.float32)
    b1a[:_HID, 0] = np.asarray(b1, np.float32)
    b1a[_HID, 0] = 1.0
    w2a = np.zeros((_HID + 1, _CLS), bf)
    w2a[:_HID] = np.asarray(W2, np.float32).astype(bf)
    w2a[_HID] = np.asarray(b2, np.float32).astype(bf)
    return x_bf, w1p, b1a, w2a


def _core_x(x_bf, c):
    """Per-core input in device layout [blk, p, kc, r] (4000B runs)."""
    xs = x_bf[c * _RPC : (c + 1) * _RPC]  # [12500, 512]
    # row = blk*500 + r ; feature = kc*128 + p
    return np.ascontiguousarray(
        xs.reshape(_NBLK, _BLK, _KC, _P).transpose(0, 3, 2, 1)
    )


def _in_maps(x, W1, b1, W2, b2):
    x_bf, w1p, b1a, w2a = _host_prep(x, W1, b1, W2, b2)
    return [
        {"xt": _core_x(x_bf, c), "w1": w1p, "b1": b1a, "w2": w2a}
        for c in range(_NCORES)
    ]


def _unshard(res):
    outs = []
    lps = []
    for c in range(_NCORES):
        a = np.asarray(res.results[c]["both"])[:, :_SUB].astype(np.float32)
        l = np.asarray(res.results[c]["last"])[:_SUB].astype(np.float32)
        lse = np.asarray(res.results[c]["lse"])[:_SUB].astype(np.float32)
        # a[q, p, kq, si, c] -> rows (q, kq, si, p)
        la = a.transpose(0, 2, 3, 1, 4).reshape(_NQUAD * 4 * _BLK, _CLS)
        # l[p, si, c] -> rows (si, p)
        ll = l.transpose(1, 0, 2).reshape(_BLK, _CLS)
        lp = np.concatenate([la, ll])  # [12500, 40] logp
        # lse[p, b, si] -> row b*500 + si*125 + p
        lse_rows = lse.transpose(1, 2, 0).reshape(_RPC)
        out = lp + lse_rows[:, None]
        lps.append(lp)
        outs.append(out)
    return np.concatenate(lps), np.concatenate(outs)


def _bern_prop_host(h, edge_index, theta):
    """Fallback: full Bernstein propagation on host (only if temp != ones)."""
    from math import comb

    n = h.shape[0]
    src = np.asarray(edge_index[0], np.int64)
    dst = np.asarray(edge_index[1], np.int64)
    deg = np.bincount(src, minlength=n).astype(np.float32)
    dis = np.where(deg > 0, 1.0 / np.sqrt(np.maximum(deg, 1.0)), 0.0).astype(
        np.float32
    )

    def anorm(v):
        msg = v[src] * dis[src][:, None]
        out = np.zeros_like(v)
        np.add.at(out, dst, msg)
        return out * dis[:, None]

    K = len(theta) - 1
    tmp = [h]
    for _ in range(K):
        t = tmp[-1]
        tmp.append(t + anorm(t))
    c = np.array([comb(K, j) / 2.0**K for j in range(K + 1)], np.float32)
    acc = np.zeros_like(h)
    for j in range(K, 0, -1):
        s = acc + c[j] * theta[j] * tmp[K - j]
        acc = s - anorm(s)
    return c[0] * theta[0] * tmp[K] + acc


def kernel(x, edge_index, W1, b1, W2, b2, temp):
    from concourse.bass_utils import run_bass_kernel_spmd

    nc = _get_bass()
    in_maps = _in_maps(x, W1, b1, W2, b2)
    res = run_bass_kernel_spmd(nc, in_maps, core_ids=list(range(_NCORES)))
    lp, out = _unshard(res)

    theta = np.maximum(np.asarray(temp, np.float32), 0.0)
    if not np.allclose(theta, 1.0):
        # General-temp path: device computed h; propagate on host, then
        # recompute log_softmax.
        out = _bern_prop_host(out.astype(np.float32), edge_index, theta)
        m = out.max(axis=1, keepdims=True)
        lp = out - (np.log(np.exp(out - m).sum(axis=1, keepdims=True)) + m)
        lp = lp.astype(np.float32)

    return lp, out


# revision 28
# speedup vs baseline: 1.4953x; 1.4840x over previous
"""Trainium2 kernel for nn_BernNet_47364899340878.

Math note (why the device kernel is just the MLP):
  The reference computes  out = sum_{j=0..K} c_j * relu(temp_j) * L^j (2I-L)^{K-j} h
  with c_j = C(K,j)/2^K and h = relu(x@W1+b1)@W2+b2.  The graded inputs pin
  temp = ones (spec fill "ones"), so relu(temp_j) = 1 for all j.  L and
  (2I - L) are commuting polynomials in the normalized adjacency, so the
  binomial theorem gives

      sum_j C(K,j) L^j (2I-L)^{K-j} = (L + 2I - L)^K = (2I)^K = 2^K I,

  i.e. the whole K=10 Bernstein propagation is exactly the identity map and
  out == h.  A non-ones temp (never the case for the graded inputs) falls
  back to a host implementation of the propagation for correctness.

Device kernel: h = relu(x@W1+b1)@W2+b2 and log_softmax(h), row-sharded over
8 NeuronCores (12500 rows each).  The kernel is HBM-bandwidth bound; traffic
per core is ~12.8 MB in + ~1.1 MB out (~39 us roofline at 358 GB/s):
  - every DMA uses ALL 128 SBUF partitions: profiling showed the HWDGE
    splits a transfer across SDMA engines by dividing the partition count
    evenly (largest divisor <= 16), so 125-partition transfers ran on only
    5 of 16 engines (111 GB/s ceiling) while 128-partition ones use all 16
    (~300-360 GB/s).  The contraction is host-padded 500 -> 512 = 4 x 128,
  - x streams as bf16, two 500-row blocks per DMA, in a host-prepped layout
    [pair, p(128), kc(4), r(1000)] (8000B contiguous run per partition),
  - only logp ships from the device (bf16, 4 blocks per DMA), plus one tiny
    fp32 lse tensor at the end; the host reconstructs the raw logits as
    out = logp + lse exactly,
  - compute runs as a per-block software pipeline S1 mm1 -> S2 bias+relu ->
    S3 mm2 -> S4 softmax-epilogue with S2/S3 one block behind S1 and S4 two
    behind, so each engine's FIFO always has the PE-gating work first,
  - mm1 accumulates into two 250-column PSUM half-tiles in separate banks;
    the bias+relu halves run on DIFFERENT engines (ACT Relu-with-bias for
    half 1, DVE tensor_scalar for half 2) and write separate SBUF tiles so
    the PE's mm2 subtiles never wait on the busier engine,
  - logp output DMAs issue from the otherwise-idle GpSimd (SWDGE) so their
    ~1us issue cost never delays ACT's relu/exp queue,
  - Exp, Ln and Relu are pinned to their shared ACT table set so the whole
    kernel does one table load,
  - DMA/compute startup is ordered so the first matmul's semaphore wait
    (which covers every earlier-emitted input DMA) only gates on w1 + the
    leftover block: w1, xl, small weights, 3 HAM warm-up matmuls, block 24's
    matmuls, and only then the streaming pair DMAs.
Bias handling folds into the matmuls: W1 gains a 65th output column of
zeros whose bias is 1.0 so h^T gets a row of ones, and W2 gains a 65th
input row equal to b2.  (If b2 != 0 AND temp != ones the host fallback
recomputes exactly; for graded inputs b2 = 0.)
Numeric error vs the fp32 reference is ~7e-3 absmax-rel (gate 2e-2).
"""

import numpy as np

_N = 100000
_FIN = 500
_FPAD = 512  # contraction padded to 4 chunks x 128 partitions
_HID = 64
_CLS = 40
_NCORES = 8
_RPC = _N // _NCORES  # 12500 rows per core
_P = 128  # contraction partitions per chunk
_KC = 4  # contraction chunks
_BLK = 500  # rows per block
_NBLK = _RPC // _BLK  # 25
_NPAIR = 12  # paired input DMAs; block 24 is the leftover
_NQUAD = 6  # output DMAs of 4 blocks each (blocks 0..23)
_SUB = 125  # rows per mm2 subtile
_NSUB = 4
_HPAD = 504  # h^T tile columns: 500 rows + 4 zero pad (128-col mm2 slices)

_CACHE = {}


def _build_bass():
    """Build the per-core Bass program (shared by all 8 cores)."""
    from contextlib import ExitStack

    import concourse.bacc as bacc
    import concourse.mybir as mybir
    import concourse.tile as tile

    fp32 = mybir.dt.float32
    bf16 = mybir.dt.bfloat16
    AF = mybir.ActivationFunctionType
    OP = mybir.AluOpType

    # Bacc (not plain Bass): its compile() runs move_matmul_waits_to_ldweights
    # + generate_event_semaphores, which split excess on_wait entries to meet
    # TRN2's 1-wait-per-instruction constraint that walrus enforces.
    #
    # Table-set pinning: ACT function tables are loaded as named sets and a
    # set switch costs ~1.3-2.7us.  Exp and Ln both live in the
    # "natural_log_exp_and_others" set, but the default insertion pass picks
    # each function's first containing set, so an Exp/Ln mix reloads on every
    # switch.  Restricting Exp/Ln to their shared set (keeping every set's
    # positional id intact) makes the whole kernel need exactly one load.
    class _PinnedActBacc(bacc.Bacc):
        def insert_act_table_loads(self):
            import bass_rust as _bass_rust
            from concourse.hw_specs import get_activation_tables

            has_activation = any(
                isinstance(i, mybir.InstActivation)
                for b in self.main_func.blocks
                for i in b.instructions
            )
            if not has_activation:
                return
            shared = {AF.Exp, AF.Ln, AF.Relu}
            tables = []
            for name, fns in get_activation_tables(self.m.arch).items():
                if name != "natural_log_exp_and_others":
                    fns = fns - shared
                tables.append((name, fns))
            _bass_rust.insert_act_table_loads(self, tables)

    nc = _PinnedActBacc()
    xt = nc.dram_tensor("xt", [_NPAIR, _P, _KC, 2 * _BLK], bf16, kind="ExternalInput")
    xl = nc.dram_tensor("xl", [_P, _KC, _BLK], bf16, kind="ExternalInput")
    w1 = nc.dram_tensor("w1", [_P, _KC, 128], bf16, kind="ExternalInput")
    b1 = nc.dram_tensor("b1", [_HID + 1, 1], fp32, kind="ExternalInput")
    w2 = nc.dram_tensor("w2", [_HID + 1, _CLS], bf16, kind="ExternalInput")
    # logp quads: [quad, p, kq(block-in-quad), si, c] bf16 — each partition's
    # quad data is one contiguous 1280B DRAM run, split into 640B descriptors.
    both = nc.dram_tensor(
        "both", [_NQUAD, _P, 4, _NSUB, _CLS], bf16, kind="ExternalOutput"
    )
    last = nc.dram_tensor("last", [_P, _NSUB, _CLS], bf16, kind="ExternalOutput")
    lse_d = nc.dram_tensor("lse", [_P, _NBLK, _NSUB], fp32, kind="ExternalOutput")

    xt_r = xt.rearrange("pr p kc r -> pr p kc r")
    both_r = both.rearrange("q p k si c -> q p k si c")

    with tile.TileContext(nc) as tc, ExitStack() as ctx:
        const = ctx.enter_context(tc.tile_pool(name="const", bufs=1))
        xpool = ctx.enter_context(tc.tile_pool(name="xin", bufs=4))
        hpool = ctx.enter_context(tc.tile_pool(name="hrelu", bufs=3))
        epool = ctx.enter_context(tc.tile_pool(name="expv", bufs=3))
        cpool = ctx.enter_context(tc.tile_pool(name="outs", bufs=2))
        spool = ctx.enter_context(tc.tile_pool(name="sums", bufs=3))
        pp1a = ctx.enter_context(tc.tile_pool(name="ps1a", bufs=2, space="PSUM"))
        pp1b = ctx.enter_context(tc.tile_pool(name="ps1b", bufs=2, space="PSUM"))
        pp2 = ctx.enter_context(tc.tile_pool(name="ps2", bufs=3, space="PSUM"))
        ppw = ctx.enter_context(tc.tile_pool(name="psw", bufs=1, space="PSUM"))

        # Issue order matters doubly here: the SP sequencer takes ~1us to
        # issue each DMA, and the first matmul's semaphore wait covers every
        # input DMA emitted before it.  So: w1 first (it lands ~1.7us before
        # xl and unblocks the warm-up matmuls), then xl (block 24's data),
        # then w2/b1 (needed one pipeline stage later); the streaming pair
        # DMAs are emitted only after block 24's matmuls.
        w1_sb = const.tile([_P, _KC, 128], bf16)
        nc.sync.dma_start(w1_sb[:], w1[:])
        xl_sb = xpool.tile([_P, _KC, _BLK], bf16, tag="xl")
        nc.sync.dma_start(xl_sb[:], xl[:])
        w2_sb = const.tile([_HID + 1, _CLS], bf16)
        nc.sync.dma_start(w2_sb[:], w2[:])
        b1_sb = const.tile([_HID + 1, 1], fp32)
        nc.sync.dma_start(b1_sb[:], b1[:])
        lse_sb = const.tile([_P, _NBLK, _NSUB], fp32)
        pair_sb = {}

        # ~2us of dummy matmuls gated only on w1 (which lands ~1.7us before
        # xl): the PE would otherwise sit idle, and this pre-warms the HAM
        # clock before block 24's real matmuls start.
        warm_ps = ppw.tile([128, _KC, 128], fp32)
        for i in range(3):
            nc.tensor.matmul(warm_ps[:], w1_sb[:, 0, :], w1_sb[:])



        def mm1_block(xt_sb, k):
            # h^T = (W1p^T @ x^T) : [128(65 live), 500], accumulated over 4
            # K-chunks, split into two 250-row column halves in SEPARATE PSUM
            # banks so the bias+relu of half 1 can run while half 2's matmuls
            # are still streaming (PSUM bank collision rules forbid reading a
            # bank the PE is writing).
            h1 = pp1a.tile([128, _BLK // 2], fp32)
            h2 = pp1b.tile([128, _BLK // 2], fp32)
            for half, hp in ((0, h1), (1, h2)):
                lo = k * _BLK + half * (_BLK // 2)
                for kc in range(_KC):
                    nc.tensor.matmul(
                        hp[:],
                        w1_sb[:, kc, :],
                        xt_sb[:, kc, lo : lo + _BLK // 2],
                        start=(kc == 0),
                        stop=(kc == _KC - 1),
                    )
            return h1, h2

        def relu_block(h1, h2):
            # fused bias+relu, one half per engine (DVE tensor_scalar + ACT
            # Relu-with-bias) so neither engine's queue gates the PE's mm2;
            # row 64 = max(0+1,0) = 1 (the bias-ones row).  Relu shares the
            # pinned ACT table set with Exp/Ln, so no table reloads.  The two
            # halves are SEPARATE tiles so mm2's first subtiles depend only
            # on the DVE half, not on whichever engine finishes last.
            # ACT takes half 1 (mm2's si0/si1 need it only after the whole
            # next mm1 block, so ACT's exp/ln/dma queue can't hurt); DVE takes
            # half 2, which gates si2/si3 soonest after mm1 completes.
            r1 = hpool.tile([_HID + 1, _BLK // 2], bf16, tag="ht1")
            nc.scalar.activation(
                r1[:], h1[: _HID + 1, :], AF.Relu, bias=b1_sb[:],
            )
            r2 = hpool.tile([_HID + 1, _BLK // 2], bf16, tag="ht2")
            nc.vector.tensor_scalar(
                out=r2[:], in0=h2[: _HID + 1, :],
                scalar1=b1_sb[:], scalar2=0.0, op0=OP.add, op1=OP.max,
            )
            return r1, r2

        def mm2_block(relus):
            # out = h_relu_aug^T.T @ W2_aug : 4 subtiles of 125 rows, two per
            # relu half (LDWEIGHTS are pulled ahead by the PE reorder window,
            # so the per-subtile stationary reload is fully hidden)
            r1, r2 = relus
            o_ps = pp2.tile([_SUB, _NSUB, _CLS], fp32)
            for si in range(_NSUB):
                src_t = r1 if si < 2 else r2
                lo = (si % 2) * _SUB
                nc.tensor.matmul(
                    o_ps[:, si, :],
                    src_t[:, lo : lo + _SUB],
                    w2_sb[:],
                )
            return o_ps

        def epi_block(o_ps, b, cmb_slot):
            # log_softmax without max-subtraction (logits bounded |h| < ~6 so
            # exp cannot overflow).  exp/sub read PSUM directly.  Only lanes
            # 0..124 are live; the output tiles' lanes 125..127 were zeroed
            # once per pool slot so the 128-partition DMAs ship finite data.
            e_sb = epool.tile([_SUB, _NSUB, _CLS], fp32)
            nc.scalar.activation(e_sb[:], o_ps[:], AF.Exp)
            ssum = spool.tile([_SUB, _NSUB], fp32)
            nc.vector.tensor_reduce(
                out=ssum[:], in_=e_sb[:], op=OP.add, axis=mybir.AxisListType.X,
            )
            nc.scalar.activation(lse_sb[:_SUB, b, :], ssum[:], AF.Ln)
            nc.vector.tensor_sub(
                cmb_slot[:_SUB],
                o_ps[:],
                lse_sb[:_SUB, b, :, None].broadcast_to([_SUB, _NSUB, _CLS]),
            )

        # Software pipeline over blocks, leftover block 24 first (its data
        # lands before pair 0, and processing it first keeps the tail short).
        # Stages per block: S1 mm1 -> S2 bias+relu -> S3 mm2 -> S4 softmax
        # epilogue, with S2/S3 one block behind S1 and S4 two behind.  This
        # keeps the DVE relu (which gates the PE's mm2) ahead of the longer
        # softmax chain in the DVE FIFO, and sandwiches each mm2 between
        # mm1s so the PE never head-of-line blocks on the relu.
        seq = [_NBLK - 1] + list(range(_NBLK - 1))
        cmb_last = cpool.tile([_P, _NSUB, _CLS], bf16, tag="cl")
        nc.vector.memset(cmb_last[96:], 0.0)
        nc.vector.memset(lse_sb[96:], 0.0)
        cmb_quad = {}

        def cmb_slot(b):
            if b == _NBLK - 1:
                return cmb_last[:]
            q = b // 4
            if q not in cmb_quad:
                cmb_quad[q] = cpool.tile([_P, 4, _NSUB, _CLS], bf16, tag="cq", name=f"cmbq{q}")
                if q < 2:
                    # cpool slots cycle round-robin; zero the dead lanes of
                    # each slot once so every later quad ships finite data
                    nc.vector.memset(cmb_quad[q][96:], 0.0)
            return cmb_quad[q][:, b % 4]

        ht_ps_of = {}
        o_ps_of = {}

        def stage1(b):
            if b == _NBLK - 1:
                ht_ps_of[b] = mm1_block(xl_sb, 0)
                for pr in (0, 1):
                    t = xpool.tile(
                        [_P, _KC, 2 * _BLK], bf16, tag="xt", name=f"xtp{pr}"
                    )
                    nc.sync.dma_start(t[:], xt_r[pr])
                    pair_sb[pr] = t
                return
            pr, k = divmod(b, 2)
            # keep the input queue 2 pairs ahead of compute
            if k == 0 and pr + 2 <= _NPAIR - 1 and pr + 2 not in pair_sb:
                t = xpool.tile([_P, _KC, 2 * _BLK], bf16, tag="xt", name=f"xtp{pr + 2}")
                nc.sync.dma_start(t[:], xt_r[pr + 2])
                pair_sb[pr + 2] = t
            ht_ps_of[b] = mm1_block(pair_sb[pr], k)

        def stage23(b, idx):
            h1, h2 = ht_ps_of.pop(b)
            o_ps_of[b] = mm2_block(relu_block(h1, h2))

        def stage4(b):
            epi_block(o_ps_of.pop(b), b, cmb_slot(b))
            if b == _NBLK - 1:
                nc.gpsimd.dma_start(last[:], cmb_last[:])
            elif b % 4 == 3:
                # one DMA per quad via GpSimd SWDGE — the engine is idle, so
                # the ~1us descriptor-issue cost never delays ACT's relu/exp
                # queue; [128, 1280B] spreads all 16 SDMA engines
                nc.gpsimd.dma_start(both_r[b // 4], cmb_quad.pop(b // 4)[:])

        for idx, b in enumerate(seq):
            stage1(b)
            if idx >= 1:
                stage23(seq[idx - 1], idx - 1)
            if idx >= 2:
                stage4(seq[idx - 2])
        stage23(seq[-1], len(seq) - 1)
        stage4(seq[-2])
        stage4(seq[-1])

        # ship the lse accumulator once at the end (64KB)
        nc.gpsimd.dma_start(lse_d[:], lse_sb[:])

    nc.finalize()
    return nc


def _get_bass():
    if "nc" not in _CACHE:
        _CACHE["nc"] = _build_bass()
    return _CACHE["nc"]


def _host_prep(x, W1, b1, W2, b2):
    """Weights/bias in device layout (bf16, bias-augmented, FWL/DMA-padded)."""
    import ml_dtypes

    bf = ml_dtypes.bfloat16
    x = np.asarray(x, np.float32)
    x_bf = np.zeros((x.shape[0], _FPAD), bf)
    x_bf[:, :_FIN] = x.astype(bf)  # [N, 512]
    w1p = np.zeros((_P, _KC, 128), bf)
    W1b = np.zeros((_FPAD, _HID), bf)
    W1b[:_FIN] = np.asarray(W1, np.float32).astype(bf)
    # feature f = kc*128 + p  ->  w1p[p, kc, m]
    w1p[:, :, :_HID] = W1b.reshape(_KC, _P, _HID).transpose(1, 0, 2)
    b1a = np.zeros((_HID + 1, 1), np.float32)
    b1a[:_HID, 0] = np.asarray(b1, np.float32)
    b1a[_HID, 0] = 1.0
    w2a = np.zeros((_HID + 1, _CLS), bf)
    w2a[:_HID] = np.asarray(W2, np.float32).astype(bf)
    w2a[_HID] = np.asarray(b2, np.float32).astype(bf)
    return x_bf, w1p, b1a, w2a


def _core_x(x_bf, c):
    """Per-core inputs: paired blocks [pr, p, kc, r(1000)] + leftover block."""
    xs = x_bf[c * _RPC : (c + 1) * _RPC]  # [12500, 512]
    # row = pr*1000 + r ; feature = kc*128 + p
    xp = np.ascontiguousarray(
        xs[: _NPAIR * 2 * _BLK]
        .reshape(_NPAIR, 2 * _BLK, _KC, _P)
        .transpose(0, 3, 2, 1)
    )
    xlast = np.ascontiguousarray(
        xs[_NPAIR * 2 * _BLK :].reshape(_BLK, _KC, _P).transpose(2, 1, 0)
    )
    return xp, xlast


def _in_maps(x, W1, b1, W2, b2):
    x_bf, w1p, b1a, w2a = _host_prep(x, W1, b1, W2, b2)
    maps = []
    for c in range(_NCORES):
        xp, xlast = _core_x(x_bf, c)
        maps.append({"xt": xp, "xl": xlast, "w1": w1p, "b1": b1a, "w2": w2a})
    return maps


def _unshard(res):
    outs = []
    lps = []
    for c in range(_NCORES):
        a = np.asarray(res.results[c]["both"])[:, :_SUB].astype(np.float32)
        l = np.asarray(res.results[c]["last"])[:_SUB].astype(np.float32)
        lse = np.asarray(res.results[c]["lse"])[:_SUB].astype(np.float32)
        # a[q, p, kq, si, c] -> rows (q, kq, si, p)
        la = a.transpose(0, 2, 3, 1, 4).reshape(_NQUAD * 4 * _BLK, _CLS)
        # l[p, si, c] -> rows (si, p)
        ll = l.transpose(1, 0, 2).reshape(_BLK, _CLS)
        lp = np.concatenate([la, ll])  # [12500, 40] logp
        # lse[p, b, si] -> row b*500 + si*125 + p
        lse_rows = lse.transpose(1, 2, 0).reshape(_RPC)
        out = lp + lse_rows[:, None]
        lps.append(lp)
        outs.append(out)
    return np.concatenate(lps), np.concatenate(outs)


def _bern_prop_host(h, edge_index, theta):
    """Fallback: full Bernstein propagation on host (only if temp != ones)."""
    from math import comb

    n = h.shape[0]
    src = np.asarray(edge_index[0], np.int64)
    dst = np.asarray(edge_index[1], np.int64)
    deg = np.bincount(src, minlength=n).astype(np.float32)
    dis = np.where(deg > 0, 1.0 / np.sqrt(np.maximum(deg, 1.0)), 0.0).astype(
        np.float32
    )

    def anorm(v):
        msg = v[src] * dis[src][:, None]
        out = np.zeros_like(v)
        np.add.at(out, dst, msg)
        return out * dis[:, None]

    K = len(theta) - 1
    tmp = [h]
    for _ in range(K):
        t = tmp[-1]
        tmp.append(t + anorm(t))
    c = np.array([comb(K, j) / 2.0**K for j in range(K + 1)], np.float32)
    acc = np.zeros_like(h)
    for j in range(K, 0, -1):
        s = acc + c[j] * theta[j] * tmp[K - j]
        acc = s - anorm(s)
    return c[0] * theta[0] * tmp[K] + acc


def kernel(x, edge_index, W1, b1, W2, b2, temp):
    from concourse.bass_utils import run_bass_kernel_spmd

    nc = _get_bass()
    in_maps = _in_maps(x, W1, b1, W2, b2)
    res = run_bass_kernel_spmd(nc, in_maps, core_ids=list(range(_NCORES)))
    lp, out = _unshard(res)

    theta = np.maximum(np.asarray(temp, np.float32), 0.0)
    if not np.allclose(theta, 1.0):
        # General-temp path: device computed h; propagate on host, then
        # recompute log_softmax.
        out = _bern_prop_host(out.astype(np.float32), edge_index, theta)
        m = out.max(axis=1, keepdims=True)
        lp = out - (np.log(np.exp(out - m).sum(axis=1, keepdims=True)) + m)
        lp = lp.astype(np.float32)

    return lp, out
